# revision 2
# baseline (speedup 1.0000x reference)
"""Trainium2 Bass kernel for FFTResonanceBlock.

Math (per flattened resonator b of B=256, freq bin c of 1025, frame t of 128):
  coeffs = 0.5 + sigmoid(decay)*0.5*0.99
  mags[t]  = amp^2 * coeffs^(t+1)              = exp(lnsm + (t+1)*lc)
  phase[t] = cumsum_t(tanh(phase)*pi + tanh(dith)*noise[t])
           = (t+1)*sp + d*Sn[t]                (Sn = cumsum of const noise)
  spec = mags * exp(i*phase); frames = irfft(spec, 2048); overlap-add hop 1024.

Device strategy (8 cores, 32 resonators each, embarrassingly parallel):
  - tiles (c_chunk=128 partitions, t=128 free) per resonator
  - sin/cos via magic-number round + cody-waite reduction + Sin LUT
  - irfft as matmul against constant DFT matrices (bf16), with the
    overlap-add folded into PSUM accumulation via a t-shifted lhsT.
"""
import sys

sys.path.insert(0, "/opt/trn_rl_repo")

import numpy as np
import ml_dtypes

import concourse.bass as bass  # noqa: F401  (registers AP machinery)
import concourse.mybir as mybir
import concourse.tile as tile
from concourse import bacc, bass_utils

# ---- problem constants (hardcoded per spec) ----
N_CORES = 8
N_RES, EXPR = 64, 4
B = N_RES * EXPR          # 256 flattened resonators
BL = B // N_CORES         # 32 per core
C = 1025                  # rfft bins for window 2048
CP = 1152                 # padded to 9*128
NCH = CP // 128           # 9 c-chunks
T = 128                   # frames
W = 2048                  # window
HOP = 1024
N_SAMPLES = T * HOP       # 131072
BASE_RES = 0.5
RES_FACTOR = 0.99

PI = float(np.pi)
TWO_PI = 2.0 * np.pi
INV2PI = float(1.0 / TWO_PI)
MAGIC = float(np.float32(1.5 * 2**23))
CW1 = float(np.float32(6.28125))
CW2 = float(np.float32(TWO_PI - 6.28125))
CW3 = float(np.float32(TWO_PI - 6.28125 - float(np.float32(TWO_PI - 6.28125))))

F32 = mybir.dt.float32
BF16 = mybir.dt.bfloat16

_CACHE: dict = {}


def _constants():
    """Input-independent constants: noise cumsum (transposed), DFT mats, t-vec."""
    if "consts" in _CACHE:
        return _CACHE["consts"]
    import jax

    cpu = jax.devices("cpu")[0]
    with jax.default_device(cpu):
        noise = jax.random.uniform(
            jax.random.key(42), (B, T, C), minval=-1.0, maxval=1.0
        )
        noise = np.asarray(noise, dtype=np.float32)
    sn = np.cumsum(noise, axis=1, dtype=np.float32)        # (B, T, C)
    sn_t = np.zeros((B, CP, T), dtype=np.float32)
    sn_t[:, :C, :] = np.transpose(sn, (0, 2, 1))           # (B, C, T), pad c -> 0

    # DFT matrices for irfft(2048): frames[n] = sum_c Re[c]*Cm[c,n] + Im[c]*Sm[c,n]
    k = np.arange(CP, dtype=np.float64)[:, None]
    n = np.arange(W, dtype=np.float64)[None, :]
    ang = 2.0 * np.pi * k * n / W
    w = np.full((CP, 1), 2.0 / W)
    w[0, 0] = 1.0 / W
    w[C - 1, 0] = 1.0 / W
    w[C:, 0] = 0.0                                          # pad rows contribute 0
    cm = (w * np.cos(ang)).astype(ml_dtypes.bfloat16)       # (1152, 2048)
    smat = (-w * np.sin(ang)).astype(ml_dtypes.bfloat16)

    tb = np.broadcast_to(
        np.arange(1, T + 1, dtype=np.float32)[None, :], (128, T)
    ).copy()                                                # (128, 128): t+1

    _CACHE["consts"] = (sn_t, cm, smat, tb)
    return _CACHE["consts"]


def _build_program():
    """Build + compile the Bass program (one core's SPMD program)."""
    if "nc" in _CACHE:
        return _CACHE["nc"]

    nc = bacc.Bacc("TRN2", target_bir_lowering=False, debug=False, num_devices=1)

    sp_d = nc.dram_tensor("sp", (CP, BL), F32, kind="ExternalInput").ap()
    dd_d = nc.dram_tensor("dd", (CP, BL), F32, kind="ExternalInput").ap()
    lc_d = nc.dram_tensor("lc", (CP, BL), F32, kind="ExternalInput").ap()
    ls_d = nc.dram_tensor("ls", (CP, BL), F32, kind="ExternalInput").ap()
    sn_d = nc.dram_tensor("snt", (BL, CP, T), F32, kind="ExternalInput").ap()
    tb_d = nc.dram_tensor("tb", (128, T), F32, kind="ExternalInput").ap()
    cm_d = nc.dram_tensor("cmat", (CP, W), BF16, kind="ExternalInput").ap()
    sm_d = nc.dram_tensor("smat", (CP, W), BF16, kind="ExternalInput").ap()
    out_d = nc.dram_tensor("out", (BL, N_SAMPLES), F32, kind="ExternalOutput").ap()

    with tile.TileContext(nc) as tc:
        with (
            tc.tile_pool(name="const", bufs=1) as cpool,
            tc.tile_pool(name="sn", bufs=3) as snpool,
            tc.tile_pool(name="tmp", bufs=14) as tpool,
            tc.tile_pool(name="spec", bufs=2) as spool,
            tc.tile_pool(name="ola", bufs=3) as opool,
            tc.tile_pool(name="ps", bufs=2, space="PSUM") as ppool,
        ):
            # ---- constants into SBUF ----
            cm_t, sm_t = [], []
            for ch in range(NCH):
                ct = cpool.tile([128, W], BF16, tag=f"cm{ch}")
                nc.sync.dma_start(out=ct[:], in_=cm_d[ch * 128:(ch + 1) * 128, :])
                cm_t.append(ct)
                st = cpool.tile([128, W], BF16, tag=f"sm{ch}")
                nc.sync.dma_start(out=st[:], in_=sm_d[ch * 128:(ch + 1) * 128, :])
                sm_t.append(st)
            par_t = {}
            for name, dram in (("sp", sp_d), ("dd", dd_d), ("lc", lc_d), ("ls", ls_d)):
                tiles = []
                for ch in range(NCH):
                    t_ = cpool.tile([128, BL], F32, tag=f"{name}{ch}")
                    nc.sync.dma_start(
                        out=t_[:], in_=dram[ch * 128:(ch + 1) * 128, :]
                    )
                    tiles.append(t_)
                par_t[name] = tiles
            tb_t = cpool.tile([128, T], F32, tag="tb")
            nc.sync.dma_start(out=tb_t[:], in_=tb_d[:])
            halfpi_t = cpool.tile([128, 1], F32, tag="halfpi")
            nc.gpsimd.memset(halfpi_t[:], PI / 2)

            FD = NCH * T  # 1152

            for b in range(BL):
                # noise cumsum, (c, t) layout: tile[p, ch*T + t] = Sn[b, ch*128+p, t]
                sn_t_ = snpool.tile([128, FD], F32, tag="sn")
                nc.sync.dma_start(
                    out=sn_t_[:].rearrange("p (c t) -> p c t", c=NCH),
                    in_=sn_d[b].rearrange("(c p) t -> p c t", p=128),
                )

                p1 = tpool.tile([128, FD], F32, tag="tmp")
                acc = tpool.tile([128, FD], F32, tag="tmp")
                ee = tpool.tile([128, FD], F32, tag="tmp")
                for ch in range(NCH):
                    sl = slice(ch * T, (ch + 1) * T)
                    spc = par_t["sp"][ch][:, b:b + 1]
                    ddc = par_t["dd"][ch][:, b:b + 1]
                    lcc = par_t["lc"][ch][:, b:b + 1]
                    lsc = par_t["ls"][ch][:, b:b + 1]
                    # p1 = sp*(t+1)
                    nc.vector.tensor_scalar(
                        p1[:, sl], tb_t[:], spc, None, mybir.AluOpType.mult
                    )
                    # acc = d*Sn + p1
                    nc.vector.scalar_tensor_tensor(
                        acc[:, sl], sn_t_[:, sl], ddc, p1[:, sl],
                        mybir.AluOpType.mult, mybir.AluOpType.add,
                    )
                    # ee = lc*(t+1) + lnsm
                    nc.vector.tensor_scalar(
                        ee[:, sl], tb_t[:], lcc, lsc,
                        mybir.AluOpType.mult, mybir.AluOpType.add,
                    )

                # range reduction: k = round(acc/2pi); red = acc - k*2pi (cody-waite)
                t1 = tpool.tile([128, FD], F32, tag="tmp")
                nc.vector.tensor_scalar(
                    t1[:], acc[:], INV2PI, MAGIC,
                    mybir.AluOpType.mult, mybir.AluOpType.add,
                )
                kk = tpool.tile([128, FD], F32, tag="tmp")
                nc.vector.tensor_scalar(
                    kk[:], t1[:], MAGIC, None, mybir.AluOpType.subtract
                )
                red = tpool.tile([128, FD], F32, tag="tmp")
                nc.vector.cody_waite_cascade(red[:], acc[:], kk[:], CW1, CW2, CW3)
                # cos arg: wrap red + pi/2 back into [-pi, pi]
                redc = tpool.tile([128, FD], F32, tag="tmp")
                nc.vector.add_range_wrap(redc[:], red[:], PI / 2, PI, TWO_PI)

                sinv = tpool.tile([128, FD], F32, tag="tmp")
                nc.scalar.activation(sinv[:], red[:], mybir.ActivationFunctionType.Sin)
                cosv = tpool.tile([128, FD], F32, tag="tmp")
                nc.scalar.activation(cosv[:], redc[:], mybir.ActivationFunctionType.Sin)
                mags = tpool.tile([128, FD], F32, tag="tmp")
                nc.scalar.activation(mags[:], ee[:], mybir.ActivationFunctionType.Exp)

                # spectra (bf16), 129-wide per chunk with zero pad col for t-shift
                re_t = spool.tile([128, NCH * (T + 1)], BF16, tag="re")
                im_t = spool.tile([128, NCH * (T + 1)], BF16, tag="im")
                nc.gpsimd.memset(re_t[:], 0.0)
                nc.gpsimd.memset(im_t[:], 0.0)
                strided = lambda tl: tl[:].rearrange(
                    "p (c t) -> p c t", c=NCH
                )  # noqa: E731
                re_w = re_t[:].rearrange("p (c t) -> p c t", c=NCH)[:, :, 1:]
                im_w = im_t[:].rearrange("p (c t) -> p c t", c=NCH)[:, :, 1:]
                src3 = lambda tl: tl[:].rearrange("p (c t) -> p c t", c=NCH)  # noqa: E731
                nc.vector.tensor_tensor(
                    re_w, src3(mags), src3(cosv), mybir.AluOpType.mult
                )
                nc.vector.tensor_tensor(
                    im_w, src3(mags), src3(sinv), mybir.AluOpType.mult
                )

                # irfft + overlap-add fused in PSUM:
                # out[t, r] = sum_c Re[c,t]C[c,r] + Im[c,t]S[c,r]
                #           + Re[c,t-1]C[c,1024+r] + Im[c,t-1]S[c,1024+r]
                ps = ppool.tile([128, HOP], F32, tag="ps")
                n_mm = NCH * 2 * 2
                for j in range(2):          # two 512-wide PSUM banks
                    idx = 0
                    for ch in range(NCH):
                        base = ch * (T + 1)
                        for spec_t, mat_t in ((re_t, cm_t[ch]), (im_t, sm_t[ch])):
                            cur = spec_t[:, base + 1: base + 1 + T]
                            shf = spec_t[:, base: base + T]
                            nc.tensor.matmul(
                                ps[:, j * 512:(j + 1) * 512],
                                cur, mat_t[:, j * 512:(j + 1) * 512],
                                start=(idx == 0), stop=False,
                            )
                            idx += 1
                            nc.tensor.matmul(
                                ps[:, j * 512:(j + 1) * 512],
                                shf, mat_t[:, HOP + j * 512: HOP + (j + 1) * 512],
                                start=False, stop=(idx == n_mm - 1),
                            )
                            idx += 1

                ola = opool.tile([128, HOP], F32, tag="ola")
                nc.scalar.copy(ola[:], ps[:])
                nc.sync.dma_start(
                    out=out_d[b].rearrange("(t r) -> t r", t=T), in_=ola[:]
                )

    nc.compile()
    _CACHE["nc"] = nc
    return nc


def _prep_inputs(amp, phase, decay, phase_dither):
    """Host prep: flatten, derive per-(b,c) scalars, pad, transpose to (CP, B)."""

    def flat(x):
        return np.transpose(np.asarray(x, np.float32), (0, 2, 1)).reshape(B, C)

    amp_f, phase_f, decay_f, dith_f = map(flat, (amp, phase, decay, phase_dither))
    coeffs = BASE_RES + (1.0 / (1.0 + np.exp(-decay_f))) * (1.0 - BASE_RES) * RES_FACTOR
    lc = np.log(coeffs).astype(np.float32)
    lnsm = (2.0 * np.log(np.maximum(amp_f, 1e-30))).astype(np.float32)
    sp = (np.tanh(phase_f) * np.pi).astype(np.float32)
    dd = np.tanh(dith_f).astype(np.float32)

    def padT(x, padval):
        o = np.full((CP, B), padval, dtype=np.float32)
        o[:C, :] = x.T
        return o

    return padT(sp, 1.0), padT(dd, 0.5), padT(lc, -0.2), padT(lnsm, -1.4)


def kernel(amp, phase, decay, phase_dither):
    sn_t, cm, smat, tb = _constants()
    sp, dd, lc, ls = _prep_inputs(amp, phase, decay, phase_dither)
    nc = _build_program()

    in_maps = []
    for core in range(N_CORES):
        bs = slice(core * BL, (core + 1) * BL)
        in_maps.append({
            "sp": np.ascontiguousarray(sp[:, bs]),
            "dd": np.ascontiguousarray(dd[:, bs]),
            "lc": np.ascontiguousarray(lc[:, bs]),
            "ls": np.ascontiguousarray(ls[:, bs]),
            "snt": np.ascontiguousarray(sn_t[bs]),
            "tb": tb,
            "cmat": cm,
            "smat": smat,
        })

    res = bass_utils.run_bass_kernel_spmd(
        nc, in_maps, core_ids=list(range(N_CORES))
    )
    out = np.concatenate([r["out"] for r in res.results], axis=0)  # (256, 131072)
    return out.reshape(1, 1, N_RES, EXPR, N_SAMPLES)


# revision 5
# speedup vs baseline: 1.0815x; 1.0815x over previous
"""Trainium2 Bass kernel for FFTResonanceBlock.

Math (per flattened resonator b of B=256, freq bin c of 1025, frame t of 128):
  coeffs = 0.5 + sigmoid(decay)*0.5*0.99
  mags[t]  = amp^2 * coeffs^(t+1)              = exp(lnsm + (t+1)*lc)
  phase[t] = cumsum_t(tanh(phase)*pi + tanh(dith)*noise[t])
           = (t+1)*sp + d*Sn[t]                (Sn = cumsum of const noise)
  spec = mags * exp(i*phase); frames = irfft(spec, 2048); overlap-add hop 1024.

Device strategy (8 cores, 32 resonators each, embarrassingly parallel):
  - tiles (c_chunk=128 partitions, t=128 free) per resonator
  - sin/cos via magic-number round + cody-waite reduction + Sin LUT
  - irfft as matmul against constant DFT matrices (bf16), with the
    overlap-add folded into PSUM accumulation via a t-shifted lhsT.
"""
import sys

sys.path.insert(0, "/opt/trn_rl_repo")

import numpy as np
import ml_dtypes

import concourse.bass as bass  # noqa: F401  (registers AP machinery)
import concourse.mybir as mybir
import concourse.tile as tile
from concourse import bacc, bass_utils

# ---- problem constants (hardcoded per spec) ----
N_CORES = 8
N_RES, EXPR = 64, 4
B = N_RES * EXPR          # 256 flattened resonators
BL = B // N_CORES         # 32 per core
C = 1025                  # rfft bins for window 2048
CP = 1152                 # padded to 9*128
NCH = CP // 128           # 9 c-chunks
T = 128                   # frames
W = 2048                  # window
HOP = 1024
N_SAMPLES = T * HOP       # 131072
BASE_RES = 0.5
RES_FACTOR = 0.99

PI = float(np.pi)
TWO_PI = 2.0 * np.pi
INV2PI = float(1.0 / TWO_PI)
MAGIC = float(np.float32(1.5 * 2**23))
CW1 = float(np.float32(6.28125))
CW2 = float(np.float32(TWO_PI - 6.28125))
CW3 = float(np.float32(TWO_PI - 6.28125 - float(np.float32(TWO_PI - 6.28125))))

F32 = mybir.dt.float32
BF16 = mybir.dt.bfloat16

_CACHE: dict = {}


def _constants():
    """Input-independent constants: noise cumsum (transposed), DFT mats, t-vec."""
    if "consts" in _CACHE:
        return _CACHE["consts"]
    import jax

    cpu = jax.devices("cpu")[0]
    with jax.default_device(cpu):
        noise = jax.random.uniform(
            jax.random.key(42), (B, T, C), minval=-1.0, maxval=1.0
        )
        noise = np.asarray(noise, dtype=np.float32)
    sn = np.cumsum(noise, axis=1, dtype=np.float32)        # (B, T, C)
    sn_t = np.zeros((B, CP, T), dtype=np.float32)
    sn_t[:, :C, :] = np.transpose(sn, (0, 2, 1))           # (B, C, T), pad c -> 0

    # DFT matrices for irfft(2048): frames[n] = sum_c Re[c]*Cm[c,n] + Im[c]*Sm[c,n]
    k = np.arange(CP, dtype=np.float64)[:, None]
    n = np.arange(W, dtype=np.float64)[None, :]
    ang = 2.0 * np.pi * k * n / W
    w = np.full((CP, 1), 2.0 / W)
    w[0, 0] = 1.0 / W
    w[C - 1, 0] = 1.0 / W
    w[C:, 0] = 0.0                                          # pad rows contribute 0
    cm = (w * np.cos(ang)).astype(ml_dtypes.bfloat16)       # (1152, 2048)
    smat = (-w * np.sin(ang)).astype(ml_dtypes.bfloat16)

    tb = np.broadcast_to(
        np.arange(1, T + 1, dtype=np.float32)[None, :], (128, T)
    ).copy()                                                # (128, 128): t+1

    _CACHE["consts"] = (sn_t, cm, smat, tb)
    return _CACHE["consts"]


def _build_program():
    """Build + compile the Bass program (one core's SPMD program)."""
    if "nc" in _CACHE:
        return _CACHE["nc"]

    nc = bacc.Bacc("TRN2", target_bir_lowering=False, debug=False, num_devices=1)

    sp_d = nc.dram_tensor("sp", (CP, BL), F32, kind="ExternalInput").ap()
    dd_d = nc.dram_tensor("dd", (CP, BL), F32, kind="ExternalInput").ap()
    lc_d = nc.dram_tensor("lc", (CP, BL), F32, kind="ExternalInput").ap()
    ls_d = nc.dram_tensor("ls", (CP, BL), F32, kind="ExternalInput").ap()
    sn_d = nc.dram_tensor("snt", (BL, CP, T), F32, kind="ExternalInput").ap()
    tb_d = nc.dram_tensor("tb", (128, T), F32, kind="ExternalInput").ap()
    cm_d = nc.dram_tensor("cmat", (CP, W), BF16, kind="ExternalInput").ap()
    sm_d = nc.dram_tensor("smat", (CP, W), BF16, kind="ExternalInput").ap()
    out_d = nc.dram_tensor("out", (BL, N_SAMPLES), F32, kind="ExternalOutput").ap()

    with tile.TileContext(nc) as tc:
        with (
            tc.tile_pool(name="const", bufs=1) as cpool,
            tc.tile_pool(name="sn", bufs=3) as snpool,
            tc.tile_pool(name="tmp", bufs=14) as tpool,
            tc.tile_pool(name="spec", bufs=3) as spool,
            tc.tile_pool(name="ola", bufs=3) as opool,
            tc.tile_pool(name="ps", bufs=4, space="PSUM") as ppool,
        ):
            # ---- constants into SBUF ----
            # Small params first so the b=0 spec chain isn't queued behind
            # the 9.4MB of DFT matrices; DFT matrices go on the gpsimd
            # (SWDGE) queue to overlap with the sync (HWDGE) traffic.
            par_t = {}
            for name, dram in (("sp", sp_d), ("dd", dd_d), ("lc", lc_d), ("ls", ls_d)):
                tiles = []
                for ch in range(NCH):
                    t_ = cpool.tile([128, BL], F32, tag=f"{name}{ch}")
                    nc.sync.dma_start(
                        out=t_[:], in_=dram[ch * 128:(ch + 1) * 128, :]
                    )
                    tiles.append(t_)
                par_t[name] = tiles
            tb_t = cpool.tile([128, T], F32, tag="tb")
            nc.sync.dma_start(out=tb_t[:], in_=tb_d[:])
            cm_t, sm_t = [], []
            for ch in range(NCH):
                ct = cpool.tile([128, W], BF16, tag=f"cm{ch}")
                nc.gpsimd.dma_start(out=ct[:], in_=cm_d[ch * 128:(ch + 1) * 128, :])
                cm_t.append(ct)
                if ch < NCH - 1:  # Nyquist/pad chunk of the sin matrix is all-zero
                    st = cpool.tile([128, W], BF16, tag=f"sm{ch}")
                    nc.gpsimd.dma_start(out=st[:], in_=sm_d[ch * 128:(ch + 1) * 128, :])
                    sm_t.append(st)

            FD = NCH * T  # 1152

            for b in range(BL):
                # noise cumsum, (c, t) layout: tile[p, ch*T + t] = Sn[b, ch*128+p, t]
                sn_t_ = snpool.tile([128, FD], F32, tag="sn")
                nc.sync.dma_start(
                    out=sn_t_[:].rearrange("p (c t) -> p c t", c=NCH),
                    in_=sn_d[b].rearrange("(c p) t -> p c t", p=128),
                )

                p1 = tpool.tile([128, FD], F32, tag="tmp")
                acc = tpool.tile([128, FD], F32, tag="tmp")
                ee = tpool.tile([128, FD], F32, tag="tmp")
                for ch in range(NCH):
                    sl = slice(ch * T, (ch + 1) * T)
                    spc = par_t["sp"][ch][:, b:b + 1]
                    ddc = par_t["dd"][ch][:, b:b + 1]
                    lcc = par_t["lc"][ch][:, b:b + 1]
                    lsc = par_t["ls"][ch][:, b:b + 1]
                    # p1 = sp*(t+1)
                    nc.vector.tensor_scalar(
                        p1[:, sl], tb_t[:], spc, None, mybir.AluOpType.mult
                    )
                    # acc = d*Sn + p1
                    nc.vector.scalar_tensor_tensor(
                        acc[:, sl], sn_t_[:, sl], ddc, p1[:, sl],
                        mybir.AluOpType.mult, mybir.AluOpType.add,
                    )
                    # ee = lc*(t+1) + lnsm
                    nc.vector.tensor_scalar(
                        ee[:, sl], tb_t[:], lcc, lsc,
                        mybir.AluOpType.mult, mybir.AluOpType.add,
                    )

                # range reduction: k = round(acc/2pi); red = acc - k*2pi (cody-waite)
                t1 = tpool.tile([128, FD], F32, tag="tmp")
                nc.vector.tensor_scalar(
                    t1[:], acc[:], INV2PI, MAGIC,
                    mybir.AluOpType.mult, mybir.AluOpType.add,
                )
                kk = tpool.tile([128, FD], F32, tag="tmp")
                nc.vector.tensor_scalar(
                    kk[:], t1[:], MAGIC, None, mybir.AluOpType.subtract
                )
                red = tpool.tile([128, FD], F32, tag="tmp")
                nc.vector.cody_waite_cascade(red[:], acc[:], kk[:], CW1, CW2, CW3)
                # cos arg: wrap red + pi/2 back into [-pi, pi]
                redc = tpool.tile([128, FD], F32, tag="tmp")
                nc.vector.add_range_wrap(redc[:], red[:], PI / 2, PI, TWO_PI)

                sinv = tpool.tile([128, FD], F32, tag="tmp")
                nc.scalar.activation(sinv[:], red[:], mybir.ActivationFunctionType.Sin)
                cosv = tpool.tile([128, FD], F32, tag="tmp")
                nc.scalar.activation(cosv[:], redc[:], mybir.ActivationFunctionType.Sin)
                mags = tpool.tile([128, FD], F32, tag="tmp")
                nc.scalar.activation(mags[:], ee[:], mybir.ActivationFunctionType.Exp)

                # spectra (bf16), 129-wide per chunk with zero pad col for t-shift
                re_t = spool.tile([128, NCH * (T + 1)], BF16, tag="re")
                im_t = spool.tile([128, NCH * (T + 1)], BF16, tag="im")
                nc.gpsimd.memset(re_t[:], 0.0)
                nc.gpsimd.memset(im_t[:], 0.0)
                strided = lambda tl: tl[:].rearrange(
                    "p (c t) -> p c t", c=NCH
                )  # noqa: E731
                re_w = re_t[:].rearrange("p (c t) -> p c t", c=NCH)[:, :, 1:]
                im_w = im_t[:].rearrange("p (c t) -> p c t", c=NCH)[:, :, 1:]
                src3 = lambda tl: tl[:].rearrange("p (c t) -> p c t", c=NCH)  # noqa: E731
                nc.vector.tensor_tensor(
                    re_w, src3(mags), src3(cosv), mybir.AluOpType.mult
                )
                nc.vector.tensor_tensor(
                    im_w, src3(mags), src3(sinv), mybir.AluOpType.mult
                )

                # irfft + overlap-add fused in PSUM:
                # out[t, r] = sum_c Re[c,t]C[c,r] + Im[c,t]S[c,r]
                #           + Re[c,t-1]C[c,1024+r] + Im[c,t-1]S[c,1024+r]
                ps = ppool.tile([128, HOP], F32, tag="ps")
                pairs = []
                for ch in range(NCH):
                    pairs.append((ch, re_t, cm_t[ch]))
                    if ch < NCH - 1:  # sin rows at Nyquist/pad chunk are zero
                        pairs.append((ch, im_t, sm_t[ch]))
                n_mm = 2 * len(pairs)
                for j in range(2):          # two 512-wide PSUM banks
                    idx = 0
                    for ch, spec_t, mat_t in pairs:
                        base = ch * (T + 1)
                        cur = spec_t[:, base + 1: base + 1 + T]
                        shf = spec_t[:, base: base + T]
                        nc.tensor.matmul(
                            ps[:, j * 512:(j + 1) * 512],
                            cur, mat_t[:, j * 512:(j + 1) * 512],
                            start=(idx == 0), stop=False,
                        )
                        idx += 1
                        nc.tensor.matmul(
                            ps[:, j * 512:(j + 1) * 512],
                            shf, mat_t[:, HOP + j * 512: HOP + (j + 1) * 512],
                            start=False, stop=(idx == n_mm - 1),
                        )
                        idx += 1

                ola = opool.tile([128, HOP], F32, tag="ola")
                nc.scalar.copy(ola[:], ps[:])
                nc.sync.dma_start(
                    out=out_d[b].rearrange("(t r) -> t r", t=T), in_=ola[:]
                )

    nc.compile()
    _CACHE["nc"] = nc
    return nc


def _prep_inputs(amp, phase, decay, phase_dither):
    """Host prep: flatten, derive per-(b,c) scalars, pad, transpose to (CP, B)."""

    def flat(x):
        return np.transpose(np.asarray(x, np.float32), (0, 2, 1)).reshape(B, C)

    amp_f, phase_f, decay_f, dith_f = map(flat, (amp, phase, decay, phase_dither))
    coeffs = BASE_RES + (1.0 / (1.0 + np.exp(-decay_f))) * (1.0 - BASE_RES) * RES_FACTOR
    lc = np.log(coeffs).astype(np.float32)
    lnsm = (2.0 * np.log(np.maximum(amp_f, 1e-30))).astype(np.float32)
    sp = (np.tanh(phase_f) * np.pi).astype(np.float32)
    dd = np.tanh(dith_f).astype(np.float32)

    def padT(x, padval):
        o = np.full((CP, B), padval, dtype=np.float32)
        o[:C, :] = x.T
        return o

    return padT(sp, 1.0), padT(dd, 0.5), padT(lc, -0.2), padT(lnsm, -1.4)


def kernel(amp, phase, decay, phase_dither):
    sn_t, cm, smat, tb = _constants()
    sp, dd, lc, ls = _prep_inputs(amp, phase, decay, phase_dither)
    nc = _build_program()

    in_maps = []
    for core in range(N_CORES):
        bs = slice(core * BL, (core + 1) * BL)
        in_maps.append({
            "sp": np.ascontiguousarray(sp[:, bs]),
            "dd": np.ascontiguousarray(dd[:, bs]),
            "lc": np.ascontiguousarray(lc[:, bs]),
            "ls": np.ascontiguousarray(ls[:, bs]),
            "snt": np.ascontiguousarray(sn_t[bs]),
            "tb": tb,
            "cmat": cm,
            "smat": smat,
        })

    res = bass_utils.run_bass_kernel_spmd(
        nc, in_maps, core_ids=list(range(N_CORES))
    )
    out = np.concatenate([r["out"] for r in res.results], axis=0)  # (256, 131072)
    return out.reshape(1, 1, N_RES, EXPR, N_SAMPLES)
